# revision 1
# baseline (speedup 1.0000x reference)
"""Trainium2 Bass kernel for nn_DeformConv2d (B=16, Cin=Cout=64, H=W=64, K=3).

Strategy (data-parallel over batch, 2 images per core on 8 cores):
  1. PE: offset conv (9 accumulating matmuls per image, K=64, M=18).
  2. DVE: bilinear "tent" coefficients tent(delta - D), window D in
     {-1,0,1}^2 around each tap, in compact [81, 4096] per-image layout.
     For |delta|<1 this reproduces bilinear sampling exactly; zero image
     padding reproduces the reference boundary handling exactly.
  3. DMA: coefficient maps broadcast-replicated across the 64 channel
     partitions via a DRAM round-trip (stride-0 partition source APs).
  4. DVE: 81-term shifted-window multiply-accumulate builds the im2col
     tensor cols_k per tap (no gather anywhere).
  5. PE: main conv = 9 accumulating matmuls (K=64, M=64) per image into
     one [128, 4096] f32 PSUM tile; ACT adds bias and writes f32 out.

On-chip compute is fp16 (DVE 2x mode; PSUM accumulates in f32).
kernel() accepts FULL inputs and returns the FULL [16,64,64,64] output.
"""

import numpy as np
from contextlib import ExitStack

N_CORES = 8
B, CIN, COUT, H, W = 16, 64, 64, 64, 64
KK = 9  # 3x3 taps
HW = H * W  # 4096
PADR, PADC = 2, 2
HP, WP = H + 2 * PADR, W + 2 * PADC  # 68, 68
IMG_PER_CORE = B // N_CORES  # 2
NT = 8  # matmul N tiles
NTS = HW // NT  # 512

_cache = {}


def _build_program():
    import concourse.bass as bass  # noqa: F401
    import concourse.mybir as mybir
    import concourse.tile as tile
    from concourse import bacc

    fp16 = mybir.dt.float16
    f32 = mybir.dt.float32
    AOp = mybir.AluOpType

    nc = bacc.Bacc("TRN2", target_bir_lowering=False, debug=False,
                   num_devices=N_CORES)

    xp_ext = nc.declare_dram_parameter("xp", [128, HP * WP], fp16, isOutput=False)
    woff_ext = nc.declare_dram_parameter("woff", [KK, CIN, 18], fp16, isOutput=False)
    wdcn_ext = nc.declare_dram_parameter("wdcn", [KK, CIN, COUT], fp16, isOutput=False)
    boff_ext = nc.declare_dram_parameter("boff", [64, 1], f32, isOutput=False)
    bdcn_ext = nc.declare_dram_parameter("bdcn", [128, 1], f32, isOutput=False)
    # per-row tent consts: c1 = 1 + D, c2 = 1 - D (D = window offset per row)
    dy1_ext = nc.declare_dram_parameter("dy1", [81, 1], f32, isOutput=False)
    dy2_ext = nc.declare_dram_parameter("dy2", [81, 1], f32, isOutput=False)
    dx1_ext = nc.declare_dram_parameter("dx1", [81, 1], f32, isOutput=False)
    dx2_ext = nc.declare_dram_parameter("dx2", [81, 1], f32, isOutput=False)
    out_ext = nc.declare_dram_parameter("out", [128, HW], f32, isOutput=True)

    offs_dram = nc.dram_tensor("offs_dram", [64, HW], fp16)
    a_dram = nc.dram_tensor("a_dram", [2 * 81, HW], fp16)

    with tile.TileContext(nc) as tc, ExitStack() as ctx:
        pool = ctx.enter_context(tc.tile_pool(name="sbuf", bufs=1))
        tmp = ctx.enter_context(tc.tile_pool(name="tmps", bufs=2))
        dbuf = ctx.enter_context(tc.tile_pool(name="dstream", bufs=3))
        ppool = ctx.enter_context(tc.tile_pool(name="psum", bufs=1, space="PSUM"))

        # ---- inputs ----
        xp = pool.tile([128, HP * WP], fp16)
        nc.sync.dma_start(xp[:], xp_ext[:])
        xp3 = xp[:].rearrange("p (r c) -> p r c", c=WP)  # [128, 68, 68]

        # weights live on BOTH partition halves (matmul lhsT must share the
        # rhs base partition; img1 rhs starts at partition 64)
        woff = pool.tile([128, KK * 18], fp16)
        wdcn = pool.tile([128, KK * COUT], fp16)
        for h in range(2):
            nc.sync.dma_start(
                woff[h * 64 : (h + 1) * 64, :].rearrange("c (k m) -> c k m", m=18),
                woff_ext[:].rearrange("k c m -> c k m"),
            )
            nc.sync.dma_start(
                wdcn[h * 64 : (h + 1) * 64, :].rearrange("c (k m) -> c k m", m=COUT),
                wdcn_ext[:].rearrange("k c m -> c k m"),
            )
        boff = pool.tile([64, 1], f32)
        nc.sync.dma_start(boff[:], boff_ext[:])
        bdcn = pool.tile([128, 1], f32)
        nc.sync.dma_start(bdcn[:], bdcn_ext[:])
        dy1 = pool.tile([81, 1], f32)
        nc.sync.dma_start(dy1[:], dy1_ext[:])
        dy2 = pool.tile([81, 1], f32)
        nc.sync.dma_start(dy2[:], dy2_ext[:])
        dx1 = pool.tile([81, 1], f32)
        nc.sync.dma_start(dx1[:], dx1_ext[:])
        dx2 = pool.tile([81, 1], f32)
        nc.sync.dma_start(dx2[:], dx2_ext[:])

        # ---- S1: offset conv ----
        # img0 rows 0-17, img1 rows 32-49 (PSUM base must be 0/32/64)
        psum_off = ppool.tile([64, HW], f32, tag="ps")
        for img in range(IMG_PER_CORE):
            for t in range(NT):
                for kk in range(KK):
                    ky, kx = kk // 3, kk % 3
                    rhs = xp3[
                        img * 64 : (img + 1) * 64,
                        (PADR - 1 + ky + 8 * t) : (PADR - 1 + ky + 8 * t + 8),
                        (PADC - 1 + kx) : (PADC - 1 + kx + W),
                    ]
                    nc.tensor.matmul(
                        psum_off[img * 32 : img * 32 + 18, t * NTS : (t + 1) * NTS],
                        woff[img * 64 : (img + 1) * 64, kk * 18 : (kk + 1) * 18],
                        rhs,
                        start=(kk == 0),
                        stop=(kk == KK - 1),
                    )

        # ---- S2: bias add + fp16 cast ----
        offs_sb = pool.tile([64, HW], fp16)
        nc.scalar.activation(
            out=offs_sb[:],
            in_=psum_off[:],
            func=mybir.ActivationFunctionType.Identity,
            bias=boff[:],
        )

        # ---- S3: offsets to DRAM (for broadcast-expansion reads) ----
        nc.sync.dma_start(offs_dram[:], offs_sb[:])

        # offs_dram rows = img*32 + 2*k + axis
        offs4 = offs_dram[:].rearrange("(a r) n -> a r n", a=2)

        # ---- S4-S6: tents and A maps per image ----
        for img in range(IMG_PER_CORE):
            tents = []
            for axis in range(2):  # 0=y, 1=x
                tin = tmp.tile([81, HW], fp16, tag="tin")
                src = (
                    offs4[img : img + 1, axis : axis + 18 : 2, :]
                    .rearrange("a k n -> (a k) n")  # [9, HW]
                    .unsqueeze(1)
                    .broadcast_to([KK, 9, HW])  # [9, 9(bcast), HW]
                )
                nc.sync.dma_start(tin[:], src)
                c1 = dy1 if axis == 0 else dx1
                c2 = dy2 if axis == 0 else dx2
                # tent(delta - D) = relu(min(1 - (delta-D), 1 + (delta-D)))
                #                 = relu(min((1+D) - delta, (1-D) + delta))
                ta = tmp.tile([81, HW], fp16, tag="t1")
                nc.vector.tensor_scalar(
                    out=ta[:], in0=tin[:], scalar1=-1.0, scalar2=c1[:],
                    op0=AOp.mult, op1=AOp.add,
                )
                tb = tmp.tile([81, HW], fp16, tag="t2")
                nc.vector.tensor_scalar(
                    out=tb[:], in0=tin[:], scalar1=c2[:], scalar2=None,
                    op0=AOp.add,
                )
                t3 = tmp.tile([81, HW], fp16, tag=f"tent{axis}")
                nc.vector.tensor_tensor(
                    out=t3[:], in0=ta[:], in1=tb[:], op=AOp.min
                )
                nc.vector.tensor_scalar(
                    out=t3[:], in0=t3[:], scalar1=0.0, scalar2=None,
                    op0=AOp.max,
                )
                tents.append(t3)
            amap = tmp.tile([81, HW], fp16, tag="amap")
            nc.vector.tensor_tensor(
                out=amap[:], in0=tents[0][:], in1=tents[1][:], op=AOp.mult
            )
            nc.sync.dma_start(a_dram[img * 81 : (img + 1) * 81, :], amap[:])

        a3 = a_dram[:].rearrange("(i r) n -> i r n", i=2)  # [2, 81, HW]

        # ---- S8: per-tap stream: A-rep DMA -> MAC -> main matmuls ----
        psum_main = ppool.tile([128, HW], f32, tag="ps")
        for kk in range(KK):
            ky, kx = kk // 3, kk % 3
            cols = dbuf.tile([128, HW], fp16, tag="cols")
            for j in range(9):
                dy, dx = j // 3 - 1, j % 3 - 1
                arep = dbuf.tile([128, HW], fp16, tag="arep")
                src = a3[:, kk * 9 + j : kk * 9 + j + 1, :].broadcast_to(
                    [2, 64, HW]
                )
                nc.sync.dma_start(arep[:], src)
                xwin = xp3[
                    :,
                    (PADR - 1 + ky + dy) : (PADR - 1 + ky + dy + H),
                    (PADC - 1 + kx + dx) : (PADC - 1 + kx + dx + W),
                ]  # [128, 64, 64]
                if j == 0:
                    nc.vector.tensor_tensor(
                        out=cols[:].rearrange("p (a b) -> p a b", b=W),
                        in0=xwin,
                        in1=arep[:].rearrange("p (a b) -> p a b", b=W),
                        op=AOp.mult,
                    )
                else:
                    prod = dbuf.tile([128, HW], fp16, tag="prod")
                    nc.vector.tensor_tensor(
                        out=prod[:].rearrange("p (a b) -> p a b", b=W),
                        in0=xwin,
                        in1=arep[:].rearrange("p (a b) -> p a b", b=W),
                        op=AOp.mult,
                    )
                    nc.vector.tensor_tensor(
                        out=cols[:], in0=cols[:], in1=prod[:], op=AOp.add
                    )
            for img in range(IMG_PER_CORE):
                for t in range(NT):
                    nc.tensor.matmul(
                        psum_main[
                            img * 64 : (img + 1) * 64, t * NTS : (t + 1) * NTS
                        ],
                        wdcn[img * 64 : (img + 1) * 64, kk * COUT : (kk + 1) * COUT],
                        cols[img * 64 : (img + 1) * 64, t * NTS : (t + 1) * NTS],
                        start=(kk == 0),
                        stop=(kk == KK - 1),
                    )

        # ---- S9: bias + f32 output ----
        out_sb = pool.tile([128, HW], f32)
        nc.scalar.activation(
            out=out_sb[:],
            in_=psum_main[:],
            func=mybir.ActivationFunctionType.Identity,
            bias=bdcn[:],
        )
        nc.sync.dma_start(out_ext[:], out_sb[:])

    nc.compile()
    return nc


def _host_prep(x, w_off, b_off, w_dcn, b_dcn):
    """Per-core input maps. numpy layout/dtype prep only."""
    fp16 = np.float16
    x = np.asarray(x, dtype=np.float32)
    w_off = np.asarray(w_off, dtype=np.float32)
    b_off = np.asarray(b_off, dtype=np.float32)
    w_dcn = np.asarray(w_dcn, dtype=np.float32)
    b_dcn = np.asarray(b_dcn, dtype=np.float32)

    # lhsT per tap: [KK, CIN, M]
    woff_l = np.ascontiguousarray(
        w_off.transpose(2, 3, 1, 0).reshape(KK, CIN, 18)
    ).astype(fp16)
    wdcn_l = np.ascontiguousarray(
        w_dcn.transpose(2, 3, 1, 0).reshape(KK, CIN, COUT)
    ).astype(fp16)

    boff_rep = np.zeros((64, 1), np.float32)
    for img in range(IMG_PER_CORE):
        boff_rep[img * 32 : img * 32 + 18, 0] = b_off
    bdcn_rep = np.tile(b_dcn, IMG_PER_CORE).reshape(128, 1).astype(np.float32)

    dy_sc = np.zeros((81, 1), np.float32)
    dx_sc = np.zeros((81, 1), np.float32)
    for k in range(KK):
        for dy in range(3):
            for dx in range(3):
                r = k * 9 + dy * 3 + dx
                dy_sc[r, 0] = dy - 1
                dx_sc[r, 0] = dx - 1
    dy1, dy2 = 1.0 + dy_sc, 1.0 - dy_sc
    dx1, dx2 = 1.0 + dx_sc, 1.0 - dx_sc

    shared = {
        "woff": woff_l,
        "wdcn": wdcn_l,
        "boff": boff_rep,
        "bdcn": bdcn_rep,
        "dy1": dy1,
        "dy2": dy2,
        "dx1": dx1,
        "dx2": dx2,
    }
    in_maps = []
    for core in range(N_CORES):
        imgs = x[core * IMG_PER_CORE : (core + 1) * IMG_PER_CORE]
        xp = np.zeros((IMG_PER_CORE, CIN, HP, WP), np.float32)
        xp[:, :, PADR : PADR + H, PADC : PADC + W] = imgs
        m = {"xp": xp.reshape(128, HP * WP).astype(fp16)}
        m.update(shared)
        in_maps.append(m)
    return in_maps


def kernel(x, w_off, b_off, w_dcn, b_dcn, _trace=False):
    from concourse.bass_utils import run_bass_kernel_spmd

    if "nc" not in _cache:
        _cache["nc"] = _build_program()
    nc = _cache["nc"]

    in_maps = _host_prep(x, w_off, b_off, w_dcn, b_dcn)
    res = run_bass_kernel_spmd(nc, in_maps, list(range(N_CORES)), trace=_trace)
    _cache["last_result"] = res

    out = np.empty((B, COUT, H, W), np.float32)
    for core in range(N_CORES):
        o = np.asarray(res.results[core]["out"], dtype=np.float32)
        out[core * IMG_PER_CORE : (core + 1) * IMG_PER_CORE] = o.reshape(
            IMG_PER_CORE, COUT, H, W
        )
    return out



# revision 6
# speedup vs baseline: 3.2205x; 3.2205x over previous
"""Trainium2 Bass kernel for nn_DeformConv2d (B=16, Cin=Cout=64, H=W=64, K=3).

Strategy (data-parallel over batch, 2 images per core on 8 cores):
  1. PE: offset conv -> per-tap per-pixel offsets (dy, dx), compact
     [18 rows/img, HW] in PSUM, streamed in [*,1024] column groups.
  2. ACT: relu(+-(psum+bias)) -> compact coefficient maps dy+/dy-/dx+/dx-
     (fp16, rows img*32 + axis*9 + kk).
  3. PE: "selection" matmuls (ones-matrix lhsT) broadcast each compact
     coefficient row across the 64 channel partitions of both images
     (no DMA broadcast: this was the 85MB/1.8ms bottleneck before).
  4. ACT: drain replicated coefficient tiles PSUM->SBUF fp16.
  5. DVE+Pool: derivative-form bilinear MAC per tap (validated exactly
     equal to bilinear gather for |delta|<1):
       cols = x0 + dx+ . DXP(0,0) - dx- . DXP(0,-1)
                 + dy+ . inner1    - dy- . inner2
       inner_r = DY(r,0) + dx+ . DXY(r,0) - dx- . DXY(r,-1)
     with DXP/DY/DXY global first/second differences of the padded image.
  6. PE: main conv = 9 accumulating matmuls per image into PSUM;
     ACT adds bias, DMA writes f32 output.

kernel() accepts FULL inputs and returns the FULL [16,64,64,64] output.
"""

import numpy as np
from contextlib import ExitStack

N_CORES = 8
B, CIN, COUT, H, W = 16, 64, 64, 64, 64
KK = 9
HW = H * W  # 4096
PADR, PADC = 2, 2
HP, WP = H + 2 * PADR, W + 2 * PADC  # 68, 68
IMG_PER_CORE = B // N_CORES  # 2
NT = 2  # MAC column tiles of 2048 (32 image rows each)
NTC = HW // NT  # 2048
ROWS_NT = H // NT  # 32

_cache = {}


def _build_program():
    import concourse.bass as bass  # noqa: F401
    import concourse.mybir as mybir
    import concourse.tile as tile
    from concourse import bacc

    fp16 = mybir.dt.float16
    f32 = mybir.dt.float32
    AOp = mybir.AluOpType
    Act = mybir.ActivationFunctionType

    nc = bacc.Bacc("TRN2", target_bir_lowering=False, debug=False,
                   num_devices=N_CORES)

    xp_ext = nc.declare_dram_parameter("xp", [128, HP * WP], fp16, isOutput=False)
    woff_ext = nc.declare_dram_parameter("woff", [KK, CIN, 18], fp16, isOutput=False)
    wdcn_ext = nc.declare_dram_parameter("wdcn", [KK, CIN, COUT], fp16, isOutput=False)
    boff_ext = nc.declare_dram_parameter("boff", [64, 1], f32, isOutput=False)
    boffn_ext = nc.declare_dram_parameter("boffn", [64, 1], f32, isOutput=False)
    bdcn_ext = nc.declare_dram_parameter("bdcn", [128, 1], f32, isOutput=False)
    sel_ext = nc.declare_dram_parameter("sel", [64, 18 * 128], fp16, isOutput=False)
    out_ext = nc.declare_dram_parameter("out", [128, HW], f32, isOutput=True)

    with tile.TileContext(nc) as tc, ExitStack() as ctx:
        pool = ctx.enter_context(tc.tile_pool(name="sbuf", bufs=1))
        cpool = ctx.enter_context(tc.tile_pool(name="cmaps", bufs=2))
        tpool = ctx.enter_context(tc.tile_pool(name="tmps", bufs=1))
        opool = ctx.enter_context(tc.tile_pool(name="outs", bufs=2))
        pmain = ctx.enter_context(tc.tile_pool(name="pmain", bufs=1, space="PSUM"))
        prep = ctx.enter_context(tc.tile_pool(name="prep", bufs=1, space="PSUM"))
        poff = ctx.enter_context(tc.tile_pool(name="poff", bufs=1, space="PSUM"))

        # ---- inputs ----
        xp = pool.tile([128, HP * WP], fp16)
        for q in range(4):
            nc.sync.dma_start(xp[q * 32:(q + 1) * 32, :], xp_ext[q * 32:(q + 1) * 32, :])
        xp3 = xp[:].rearrange("p (r c) -> p r c", c=WP)

        woff = pool.tile([128, KK * 18], fp16)
        wdcn = pool.tile([128, KK * COUT], fp16)
        for h in range(2):
            nc.sync.dma_start(
                woff[h * 64:(h + 1) * 64, :].rearrange("c (k m) -> c k m", m=18),
                woff_ext[:].rearrange("k c m -> c k m"))
            nc.sync.dma_start(
                wdcn[h * 64:(h + 1) * 64, :].rearrange("c (k m) -> c k m", m=COUT),
                wdcn_ext[:].rearrange("k c m -> c k m"))
        sel = pool.tile([64, 18 * 128], fp16)
        nc.sync.dma_start(sel[:], sel_ext[:])
        boff = pool.tile([64, 1], f32)
        nc.sync.dma_start(boff[:], boff_ext[:])
        boffn = pool.tile([64, 1], f32)
        nc.sync.dma_start(boffn[:], boffn_ext[:])
        bdcn = pool.tile([128, 1], f32)
        nc.sync.dma_start(bdcn[:], bdcn_ext[:])

        # ---- compact coefficient maps (memset garbage rows vs NaN) ----
        maps_p = pool.tile([64, HW], fp16)
        maps_n = pool.tile([64, HW], fp16)
        nc.vector.memset(maps_p[:, :], 0.0)
        nc.vector.memset(maps_n[:, :], 0.0)

        # ---- global difference tensors ----
        dxp = pool.tile([128, HP * (WP - 1)], fp16)
        dxp3 = dxp[:].rearrange("p (r c) -> p r c", c=WP - 1)
        dy = pool.tile([128, (HP - 1) * WP], fp16)
        dy3 = dy[:].rearrange("p (r c) -> p r c", c=WP)
        dxy = pool.tile([128, (HP - 1) * (WP - 1)], fp16)
        dxy3 = dxy[:].rearrange("p (r c) -> p r c", c=WP - 1)
        nc.vector.tensor_tensor(out=dxp3[:, :, :], in0=xp3[:, :, 1:],
                                in1=xp3[:, :, :WP - 1], op=AOp.subtract)
        nc.gpsimd.tensor_tensor(out=dy3[:, :, :], in0=xp3[:, 1:, :],
                                in1=xp3[:, :HP - 1, :], op=AOp.subtract)
        nc.vector.tensor_tensor(out=dxy3[:, :, :], in0=dxp3[:, 1:, :],
                                in1=dxp3[:, :HP - 1, :], op=AOp.subtract)

        # ---- offset conv in [*,1024] column groups; groups 0-1 = head ----
        def offset_group(g):
            ps = poff.tile([64, 1024], f32, tag="poff")
            for img in range(IMG_PER_CORE):
                for c2 in range(2):
                    for kk in range(KK):
                        ky, kx = kk // 3, kk % 3
                        col0 = g * 1024 + c2 * 512
                        r0 = (PADR - 1 + ky) + (col0 // W)
                        rhs = xp3[img * 64:(img + 1) * 64,
                                  r0: r0 + 8,
                                  (PADC - 1 + kx):(PADC - 1 + kx + W)]
                        nc.tensor.matmul(
                            ps[img * 32: img * 32 + 18, c2 * 512:(c2 + 1) * 512],
                            woff[img * 64:(img + 1) * 64, kk * 18:(kk + 1) * 18],
                            rhs, start=(kk == 0), stop=(kk == KK - 1))
            for img in range(IMG_PER_CORE):
                rr = img * 32
                nc.scalar.activation(
                    out=maps_p[rr:rr + 18, g * 1024:(g + 1) * 1024],
                    in_=ps[rr:rr + 18, :], func=Act.Relu, bias=boff[rr:rr + 18, :])
                nc.scalar.activation(
                    out=maps_n[rr:rr + 18, g * 1024:(g + 1) * 1024],
                    in_=ps[rr:rr + 18, :], func=Act.Relu, scale=-1.0,
                    bias=boffn[rr:rr + 18, :])

        offset_group(0)
        offset_group(1)

        # ---- MAC phase ----
        # window helpers: 3D views [128, ROWS_NT, 64] of global tensors
        def win(t3, nt, ty, tx, h, w):
            r = PADR + ty + h + nt * ROWS_NT
            c = PADC + tx + w
            return t3[:, r:r + ROWS_NT, c:c + 64]

        POOL_OPS = True  # put 3 independent products on GpSimd

        for nt in range(NT):
            pm = pmain.tile([128, NTC], f32, tag="pmain")
            for kk in range(KK):
                ty, tx = kk // 3 - 1, kk % 3 - 1
                # -- replicate 4 coefficient maps for this (nt, tap) --
                # cmapX = [cx+ | cx-], cmapY = [cy+ | cy-] each [128, 2*2048]
                cmX = cpool.tile([128, 2 * NTC], fp16, tag="cmX")
                cmY = cpool.tile([128, 2 * NTC], fp16, tag="cmY")
                for (cm, axis, sgn_i) in ((cmX, 1, 0), (cmX, 1, 1),
                                          (cmY, 0, 0), (cmY, 0, 1)):
                    src = maps_p if sgn_i == 0 else maps_n
                    s = axis * 9 + kk
                    for hh in range(2):
                        pr = prep.tile([128, 1024], f32, tag="prep")
                        for c2 in range(2):
                            col0 = nt * NTC + hh * 1024 + c2 * 512
                            nc.tensor.matmul(
                                pr[:, c2 * 512:(c2 + 1) * 512],
                                sel[0:50, s * 128:(s + 1) * 128],
                                src[0:50, col0:col0 + 512],
                                start=True, stop=True)
                        nc.scalar.activation(
                            out=cm[:, sgn_i * NTC + hh * 1024:
                                   sgn_i * NTC + (hh + 1) * 1024],
                            in_=pr[:], func=Act.Identity)
                # interleave remaining offset-conv groups behind PE slack
                if nt == 0 and kk == 2:
                    offset_group(2)
                if nt == 0 and kk == 5:
                    offset_group(3)

                cxp = cmX[:, 0:NTC].rearrange("p (r c) -> p r c", c=64)
                cxn = cmX[:, NTC:2 * NTC].rearrange("p (r c) -> p r c", c=64)

                # -- MAC: 13 DVE + (optionally) 3 Pool tensor ops --
                tm12 = tpool.tile([128, 2 * NTC], fp16, tag="tm12")
                tm12v = tm12[:].rearrange("p (s r c) -> p s r c", s=2, c=64)
                tm34 = tpool.tile([128, 2 * NTC], fp16, tag="tm34")
                tm34v = tm34[:].rearrange("p (s r c) -> p s r c", s=2, c=64)
                in12 = tpool.tile([128, 2 * NTC], fp16, tag="in12")
                t5 = tpool.tile([128, NTC], fp16, tag="t5")
                t5v = t5[:].rearrange("p (r c) -> p r c", c=64)
                t6 = tpool.tile([128, NTC], fp16, tag="t6")
                t6v = t6[:].rearrange("p (r c) -> p r c", c=64)
                t78 = tpool.tile([128, 2 * NTC], fp16, tag="t78")
                cols = tpool.tile([128, NTC], fp16, tag="cols", bufs=2)

                # F1: tm12 = cx_p (x2) * [DXY(0,0) | DXY(-1,0)]
                nc.vector.tensor_tensor(
                    out=tm12v[:, 0], in0=cxp[:, :, :],
                    in1=win(dxy3, nt, ty, tx, 0, 0), op=AOp.mult)
                nc.vector.tensor_tensor(
                    out=tm12v[:, 1], in0=cxp[:, :, :],
                    in1=win(dxy3, nt, ty, tx, -1, 0), op=AOp.mult)
                # F2: tm12 += [DY(0,0) | DY(-1,0)]
                nc.vector.tensor_tensor(
                    out=tm12v[:, 0], in0=tm12v[:, 0],
                    in1=win(dy3, nt, ty, tx, 0, 0), op=AOp.add)
                nc.vector.tensor_tensor(
                    out=tm12v[:, 1], in0=tm12v[:, 1],
                    in1=win(dy3, nt, ty, tx, -1, 0), op=AOp.add)
                # F3 (Pool): tm34 = cx_n (x2) * [DXY(0,-1) | DXY(-1,-1)]
                eng3 = nc.gpsimd if POOL_OPS else nc.vector
                eng3.tensor_tensor(
                    out=tm34v[:, 0], in0=cxn[:, :, :],
                    in1=win(dxy3, nt, ty, tx, 0, -1), op=AOp.mult)
                eng3.tensor_tensor(
                    out=tm34v[:, 1], in0=cxn[:, :, :],
                    in1=win(dxy3, nt, ty, tx, -1, -1), op=AOp.mult)
                # F4: inner12 = tm12 - tm34  [128, 2*2048]
                nc.vector.tensor_tensor(
                    out=in12[:], in0=tm12[:], in1=tm34[:], op=AOp.subtract)
                # base chain
                nc.vector.tensor_tensor(
                    out=t5v[:, :, :], in0=cxp[:, :, :],
                    in1=win(dxp3, nt, ty, tx, 0, 0), op=AOp.mult)
                nc.vector.tensor_tensor(
                    out=t5v[:, :, :], in0=t5v[:, :, :],
                    in1=win(xp3, nt, ty, tx, 0, 0), op=AOp.add)
                eng6 = nc.gpsimd if POOL_OPS else nc.vector
                eng6.tensor_tensor(
                    out=t6v[:, :, :], in0=cxn[:, :, :],
                    in1=win(dxp3, nt, ty, tx, 0, -1), op=AOp.mult)
                nc.vector.tensor_tensor(
                    out=cols[:], in0=t5[:], in1=t6[:], op=AOp.subtract)
                # vertical: t78 = [cy+ | cy-] * inner12; cols += t78[0] - t78[1]
                nc.vector.tensor_tensor(
                    out=t78[:], in0=cmY[:], in1=in12[:], op=AOp.mult)
                nc.vector.tensor_tensor(
                    out=cols[:], in0=cols[:], in1=t78[:, 0:NTC], op=AOp.add)
                nc.vector.tensor_tensor(
                    out=cols[:], in0=cols[:], in1=t78[:, NTC:2 * NTC],
                    op=AOp.subtract)

                # -- main conv --
                for img in range(IMG_PER_CORE):
                    for c4 in range(4):
                        nc.tensor.matmul(
                            pm[img * 64:(img + 1) * 64, c4 * 512:(c4 + 1) * 512],
                            wdcn[img * 64:(img + 1) * 64, kk * 64:(kk + 1) * 64],
                            cols[img * 64:(img + 1) * 64, c4 * 512:(c4 + 1) * 512],
                            start=(kk == 0), stop=(kk == KK - 1))

            ob = opool.tile([128, NTC], f32, tag="ob")
            nc.scalar.activation(out=ob[:], in_=pm[:], func=Act.Identity,
                                 bias=bdcn[:])
            nc.sync.dma_start(out_ext[:, nt * NTC:(nt + 1) * NTC], ob[:])

    nc.compile()
    return nc


def _host_prep(x, w_off, b_off, w_dcn, b_dcn):
    fp16 = np.float16
    x = np.asarray(x, dtype=np.float32)
    w_off = np.asarray(w_off, dtype=np.float32)
    b_off = np.asarray(b_off, dtype=np.float32)
    w_dcn = np.asarray(w_dcn, dtype=np.float32)
    b_dcn = np.asarray(b_dcn, dtype=np.float32)

    # offset-conv lhsT columns: m = axis*9 + kk_off -> channel c = 2*kk_off+axis
    # woff_l[t, cin, m] = w_off[c(m), cin, ty(t), tx(t)]
    woff_l = np.zeros((KK, CIN, 18), np.float32)
    for t in range(KK):
        ty, tx = t // 3, t % 3
        for m in range(18):
            axis, kko = m // 9, m % 9
            c = 2 * kko + axis
            woff_l[t, :, m] = w_off[c, :, ty, tx]
    woff_l = woff_l.astype(fp16)

    wdcn_l = np.ascontiguousarray(
        w_dcn.transpose(2, 3, 1, 0).reshape(KK, CIN, COUT)).astype(fp16)

    boff_rep = np.zeros((64, 1), np.float32)
    for img in range(IMG_PER_CORE):
        for m in range(18):
            axis, kko = m // 9, m % 9
            boff_rep[img * 32 + m, 0] = b_off[2 * kko + axis]
    boffn_rep = -boff_rep

    bdcn_rep = np.tile(b_dcn, IMG_PER_CORE).reshape(128, 1).astype(np.float32)

    # selection matrices: sel[r, s*128 + m] = 1 iff r == (m//64)*32 + s
    sel_m = np.zeros((64, 18 * 128), np.float32)
    for s in range(18):
        for m in range(128):
            r = (m // 64) * 32 + s
            sel_m[r, s * 128 + m] = 1.0
    sel_m = sel_m.astype(fp16)

    shared = {
        "woff": woff_l, "wdcn": wdcn_l, "boff": boff_rep, "boffn": boffn_rep,
        "bdcn": bdcn_rep, "sel": sel_m,
    }
    in_maps = []
    for core in range(N_CORES):
        imgs = x[core * IMG_PER_CORE:(core + 1) * IMG_PER_CORE]
        xp = np.zeros((IMG_PER_CORE, CIN, HP, WP), np.float32)
        xp[:, :, PADR:PADR + H, PADC:PADC + W] = imgs
        m = {"xp": xp.reshape(128, HP * WP).astype(fp16)}
        m.update(shared)
        in_maps.append(m)
    return in_maps


def kernel(x, w_off, b_off, w_dcn, b_dcn, _trace=False):
    from concourse.bass_utils import run_bass_kernel_spmd

    if "nc" not in _cache:
        _cache["nc"] = _build_program()
    nc = _cache["nc"]

    in_maps = _host_prep(x, w_off, b_off, w_dcn, b_dcn)
    res = run_bass_kernel_spmd(nc, in_maps, list(range(N_CORES)), trace=_trace)
    _cache["last_result"] = res

    out = np.empty((B, COUT, H, W), np.float32)
    for core in range(N_CORES):
        o = np.asarray(res.results[core]["out"], dtype=np.float32)
        out[core * IMG_PER_CORE:(core + 1) * IMG_PER_CORE] = o.reshape(
            IMG_PER_CORE, COUT, H, W)
    return out
